# revision 4
# baseline (speedup 1.0000x reference)
"""Trainium2 Bass kernel for nn_AlternatingForecastModel.

2-layer LSTM (H=512) over S=2688 steps, B=512. Odd weeks feed the model's
previous prediction back as input feature 0. Data-parallel over batch:
8 cores x 64 rows, weights replicated, scan local per core.

Layout per core/step (batch bl=64):
  gates[bl, 2048] in PSUM = lhsT.T @ rhs accumulation with lhsT = transposed
  activations (curT [34,64] / hT chunks [128,64]) and rhs = pre-transposed
  weights streamed at N=512 per matmul (fp32r: full fp32 storage, 1 cyc/row).
  Gate rows reordered host-side to [i, f, o, g] so one sigmoid covers cols
  0:1536 and tanh covers 1536:2048. Elementwise on ACT/DVE in the
  batch-partition domain; h_new transposed back with PE transposes into
  reused PSUM banks; pred = wout . h1 via M=1 matmuls giving predT [1, 64].
"""

import numpy as np

import concourse.bacc as bacc
import concourse.mybir as mybir
import concourse.tile as tile
from concourse.bass import ds
from concourse.bass_utils import run_bass_kernel_spmd
from concourse.masks import make_identity

FP32 = mybir.dt.float32
FP32R = mybir.dt.float32r
AF = mybir.ActivationFunctionType
ALU = mybir.AluOpType

B, S, F = 512, 2688, 32
H = 512
G = 4 * H
WEEK = 672
NCORES = 8
BL = B // NCORES          # 64 batch rows per core
KX = F + 2                # 34: [feat0, x1..x31, flag, ones(bias)]
U = 8                     # steps per sub-block (one x DMA)
STEPS_PER_IT = 2 * U      # 16
IT_PER_WEEK = WEEK // STEPS_PER_IT  # 42
NBLK = S // U             # 336
NIT = S // STEPS_PER_IT   # 168

_LAST_RESULTS = None


def _build(bout_val: float, trace: bool = False):
    nc = bacc.Bacc("TRN2")

    xaug_d = nc.declare_dram_parameter("xaug", [NBLK * KX, U * BL], FP32R, isOutput=False)
    whh0t_d = nc.declare_dram_parameter("whh0t", [128, 4 * G], FP32R, isOutput=False)
    wih1t_d = nc.declare_dram_parameter("wih1t", [128, 4 * G], FP32R, isOutput=False)
    whh1t_d = nc.declare_dram_parameter("whh1t", [128, 4 * G], FP32R, isOutput=False)
    wih0t_d = nc.declare_dram_parameter("wih0t", [KX, G], FP32R, isOutput=False)
    bias1_d = nc.declare_dram_parameter("bias1", [1, G], FP32R, isOutput=False)
    woutt_d = nc.declare_dram_parameter("woutt", [128, 4], FP32R, isOutput=False)
    ones_d = nc.declare_dram_parameter("ones", [1, BL], FP32R, isOutput=False)
    zeros_d = nc.declare_dram_parameter("zeros", [128, 4 * BL], FP32R, isOutput=False)
    out_d = nc.declare_dram_parameter("out", [NIT, STEPS_PER_IT * BL], FP32, isOutput=True)

    # SBUF
    whh0t = nc.alloc_sbuf_tensor("whh0t_s", [128, 4 * G], FP32R)
    wih1t = nc.alloc_sbuf_tensor("wih1t_s", [128, 4 * G], FP32R)
    whh1t = nc.alloc_sbuf_tensor("whh1t_s", [128, 4 * G], FP32R)
    wih0t = nc.alloc_sbuf_tensor("wih0t_s", [KX, G], FP32R)
    bias1 = nc.alloc_sbuf_tensor("bias1_s", [1, G], FP32R)
    woutt = nc.alloc_sbuf_tensor("woutt_s", [128, 4], FP32R)
    ones = nc.alloc_sbuf_tensor("ones_s", [1, BL], FP32R)
    ident = nc.alloc_sbuf_tensor("ident", [BL, BL], FP32)
    boutb = nc.alloc_sbuf_tensor("boutb", [1, 1], FP32)

    xbuf = [nc.alloc_sbuf_tensor(f"xbuf{a}", [KX, U * BL], FP32R) for a in (0, 1)]
    h0T = [nc.alloc_sbuf_tensor(f"h0T{p}", [128, 4 * BL], FP32R) for p in (0, 1)]
    h1T = [nc.alloc_sbuf_tensor(f"h1T{p}", [128, 4 * BL], FP32R) for p in (0, 1)]
    c0 = nc.alloc_sbuf_tensor("c0", [BL, H], FP32)
    c1 = nc.alloc_sbuf_tensor("c1", [BL, H], FP32)
    sig0 = [nc.alloc_sbuf_tensor(f"sig0{p}", [BL, 3 * H], FP32) for p in (0, 1)]
    sig1 = [nc.alloc_sbuf_tensor(f"sig1{p}", [BL, 3 * H], FP32) for p in (0, 1)]
    tg0 = [nc.alloc_sbuf_tensor(f"tg0{p}", [BL, H], FP32) for p in (0, 1)]
    tg1 = [nc.alloc_sbuf_tensor(f"tg1{p}", [BL, H], FP32) for p in (0, 1)]
    tc0 = [nc.alloc_sbuf_tensor(f"tc0{p}", [BL, H], FP32) for p in (0, 1)]
    tc1 = [nc.alloc_sbuf_tensor(f"tc1{p}", [BL, H], FP32) for p in (0, 1)]
    ta = [nc.alloc_sbuf_tensor(f"ta{p}", [BL, H], FP32) for p in (0, 1)]
    tb = [nc.alloc_sbuf_tensor(f"tb{p}", [BL, H], FP32) for p in (0, 1)]
    h0 = [nc.alloc_sbuf_tensor(f"h0{p}", [BL, H], FP32) for p in (0, 1)]
    h1 = [nc.alloc_sbuf_tensor(f"h1{p}", [BL, H], FP32) for p in (0, 1)]
    outst = nc.alloc_sbuf_tensor("outst", [1, STEPS_PER_IT * BL], FP32)
    predl = nc.alloc_sbuf_tensor("predl", [1, BL], FP32R)

    g0 = nc.alloc_psum_tensor("g0", [128, G], FP32)
    g1 = nc.alloc_psum_tensor("g1", [128, G], FP32)

    NS = G // 512  # 4 N-slices per gate vector

    def emit_step(u, xb, pred_week, pred_dst):
        """One LSTM step. u: 0..15 slot in iteration; xb: staging buffer;
        pred_dst: (tensor, col) to write predT row into (next feat0), or None."""
        par, prev = u % 2, (u + 1) % 2
        slot = (u % U) * BL
        xl = xb.ap()[0:KX, slot:slot + BL]

        # ---- gates0 = curT.T @ Wih0aug + h0_prev.T.T @ Whh0 ----
        for ns in range(NS):
            nc.tensor.matmul(g0.ap()[0:BL, ns * 512:(ns + 1) * 512],
                             xl, wih0t.ap()[:, ns * 512:(ns + 1) * 512],
                             start=True, stop=False)
        for k in range(4):
            lhs = h0T[prev].ap()[:, k * BL:(k + 1) * BL]
            for ns in range(NS):
                nc.tensor.matmul(
                    g0.ap()[0:BL, ns * 512:(ns + 1) * 512],
                    lhs, whh0t.ap()[:, k * G + ns * 512:k * G + (ns + 1) * 512],
                    start=False, stop=(k == 3))
        # ---- EW layer 0 ----
        nc.scalar.activation(sig0[par].ap(), g0.ap()[0:BL, 0:3 * H], AF.Sigmoid)
        nc.scalar.activation(tg0[par].ap(), g0.ap()[0:BL, 3 * H:4 * H], AF.Tanh)
        nc.vector.tensor_mul(ta[par].ap(), sig0[par].ap()[:, 0:H], tg0[par].ap())
        nc.vector.tensor_mul(tb[par].ap(), sig0[par].ap()[:, H:2 * H], c0.ap())
        nc.vector.tensor_add(c0.ap(), ta[par].ap(), tb[par].ap())
        nc.scalar.activation(tc0[par].ap(), c0.ap(), AF.Tanh)
        nc.vector.tensor_mul(h0[par].ap(), sig0[par].ap()[:, 2 * H:3 * H],
                             tc0[par].ap())
        # ---- transpose h0 -> h0T[par] (via psum bank 0 of g0) ----
        for k in range(4):
            nc.tensor.transpose(g0.ap()[0:128, k * BL:(k + 1) * BL],
                                h0[par].ap()[0:BL, k * 128:(k + 1) * 128],
                                ident.ap())
        nc.vector.tensor_copy(h0T[par].ap(), g0.ap()[0:128, 0:4 * BL])

        # ---- gates1 = bias1 + h0_new.T.T @ Wih1 + h1_prev.T.T @ Whh1 ----
        for ns in range(NS):
            nc.tensor.matmul(g1.ap()[0:BL, ns * 512:(ns + 1) * 512],
                             ones.ap(), bias1.ap()[:, ns * 512:(ns + 1) * 512],
                             start=True, stop=False)
        for k in range(4):
            lhs = h1T[prev].ap()[:, k * BL:(k + 1) * BL]
            for ns in range(NS):
                nc.tensor.matmul(
                    g1.ap()[0:BL, ns * 512:(ns + 1) * 512],
                    lhs, whh1t.ap()[:, k * G + ns * 512:k * G + (ns + 1) * 512],
                    start=False, stop=False)
        for k in range(4):
            lhs = h0T[par].ap()[:, k * BL:(k + 1) * BL]
            for ns in range(NS):
                nc.tensor.matmul(
                    g1.ap()[0:BL, ns * 512:(ns + 1) * 512],
                    lhs, wih1t.ap()[:, k * G + ns * 512:k * G + (ns + 1) * 512],
                    start=False, stop=(k == 3))
        # ---- EW layer 1 ----
        nc.scalar.activation(sig1[par].ap(), g1.ap()[0:BL, 0:3 * H], AF.Sigmoid)
        nc.scalar.activation(tg1[par].ap(), g1.ap()[0:BL, 3 * H:4 * H], AF.Tanh)
        nc.vector.tensor_mul(ta[par].ap(), sig1[par].ap()[:, 0:H], tg1[par].ap())
        nc.vector.tensor_mul(tb[par].ap(), sig1[par].ap()[:, H:2 * H], c1.ap())
        nc.vector.tensor_add(c1.ap(), ta[par].ap(), tb[par].ap())
        nc.scalar.activation(tc1[par].ap(), c1.ap(), AF.Tanh)
        nc.vector.tensor_mul(h1[par].ap(), sig1[par].ap()[:, 2 * H:3 * H],
                             tc1[par].ap())
        # ---- transpose h1 -> h1T[par] (psum bank 4 of g1) ----
        for k in range(4):
            nc.tensor.transpose(g1.ap()[0:128, k * BL:(k + 1) * BL],
                                h1[par].ap()[0:BL, k * 128:(k + 1) * 128],
                                ident.ap())
        nc.vector.tensor_copy(h1T[par].ap(), g1.ap()[0:128, 0:4 * BL])

        # ---- pred = wout . h1_new  -> predT [1, BL] in psum (bank 5 of g1) ----
        pps = g1.ap()[0:1, 512:512 + BL]
        for k in range(4):
            nc.tensor.matmul(pps, woutt.ap()[:, k:k + 1],
                             h1T[par].ap()[:, k * BL:(k + 1) * BL],
                             start=(k == 0), stop=(k == 3))
        nc.scalar.activation(outst.ap()[0:1, u * BL:(u + 1) * BL], pps,
                             AF.Identity, bias=boutb.ap())
        if pred_week:
            dst_t, dst_col = pred_dst
            nc.scalar.activation(dst_t.ap()[0:1, dst_col:dst_col + BL], pps,
                                 AF.Identity, bias=boutb.ap())
        else:
            nc.scalar.activation(predl.ap(), pps, AF.Identity, bias=boutb.ap())

    with tile.TileContext(nc) as tc:
        # ---- preamble: weights, constants, state init ----
        make_identity(nc, ident.ap())
        nc.gpsimd.memset(boutb.ap(), float(bout_val))
        nc.gpsimd.memset(c0.ap(), 0.0)
        nc.gpsimd.memset(c1.ap(), 0.0)
        nc.sync.dma_start(out=whh0t.ap(), in_=whh0t_d.ap())
        nc.sync.dma_start(out=wih1t.ap(), in_=wih1t_d.ap())
        nc.sync.dma_start(out=whh1t.ap(), in_=whh1t_d.ap())
        nc.sync.dma_start(out=wih0t.ap(), in_=wih0t_d.ap())
        nc.sync.dma_start(out=bias1.ap(), in_=bias1_d.ap())
        nc.sync.dma_start(out=woutt.ap(), in_=woutt_d.ap())
        nc.sync.dma_start(out=ones.ap(), in_=ones_d.ap())
        for p in (0, 1):
            nc.sync.dma_start(out=h0T[p].ap(), in_=zeros_d.ap())
            nc.sync.dma_start(out=h1T[p].ap(), in_=zeros_d.ap())

        def week_loop(week, pred_week):
            blk_base = week * WEEK // U          # sub-block index base
            it_base = week * WEEK // STEPS_PER_IT

            def body(i):
                for a in (0, 1):
                    xb = xbuf[a]
                    if pred_week:
                        nc.sync.dma_start(
                            out=xb.ap()[1:KX, :],
                            in_=xaug_d.ap()[ds((blk_base + 2 * i + a) * KX + 1,
                                               KX - 1), :])
                    else:
                        nc.sync.dma_start(
                            out=xb.ap()[0:KX, :],
                            in_=xaug_d.ap()[ds((blk_base + 2 * i + a) * KX,
                                               KX), :])
                    for u8 in range(U):
                        u = a * U + u8
                        if pred_week:
                            if u8 < U - 1:
                                pdst = (xb, (u8 + 1) * BL)
                            else:
                                pdst = (xbuf[1 - a], 0)
                        else:
                            pdst = None
                        emit_step(u, xb, pred_week, pdst)
                nc.sync.dma_start(out=out_d.ap()[ds(it_base + i, 1), :],
                                  in_=outst.ap())

            with tc.For_i(0, IT_PER_WEEK, 1,
                          hint_engines=(mybir.EngineType.PE,
                                        mybir.EngineType.Activation,
                                        mybir.EngineType.DVE)) as i:
                body(i)

        week_loop(0, False)
        # pred(671) -> feat0 slot for t=672
        nc.scalar.activation(xbuf[0].ap()[0:1, 0:BL], predl.ap(), AF.Copy)
        week_loop(1, True)
        week_loop(2, False)
        nc.scalar.activation(xbuf[0].ap()[0:1, 0:BL], predl.ap(), AF.Copy)
        week_loop(3, True)

    nc.compile()
    return nc


def _prep_inputs(x, Wih0, Whh0, bih0, bhh0, Wih1, Whh1, bih1, bhh1, Wout, bout):
    """Host-side reshapes: gate reorder to [i,f,o,g], weight transposes,
    per-core xaug staging layout."""
    f32 = np.float32
    perm = np.concatenate([np.arange(0, 512), np.arange(512, 1024),
                           np.arange(1536, 2048), np.arange(1024, 1536)])

    def wT(w):  # [G, 512] -> [128, 4*G] chunk-k at cols [G*k, G*k+G)
        t = np.ascontiguousarray(w[perm].T.astype(f32))          # [512, G]
        return np.ascontiguousarray(
            t.reshape(4, 128, G).transpose(1, 0, 2).reshape(128, 4 * G))

    whh0t = wT(Whh0)
    wih1t = wT(Wih1)
    whh1t = wT(Whh1)
    bias0 = (bih0 + bhh0)[perm].astype(f32)
    bias1 = (bih1 + bhh1)[perm].astype(f32)[None, :]
    wih0p = Wih0[perm].astype(f32)                               # [G, 33]
    wih0t = np.concatenate([wih0p.T, bias0[None, :]], axis=0)    # [34, G]
    wih0t = np.ascontiguousarray(wih0t)
    woutt = np.ascontiguousarray(Wout.reshape(4, 128).T.astype(f32))

    tw = np.arange(S) // WEEK
    mask = np.where((tw % 2 == 0) & ((tw + 1) * WEEK <= S), 0.0, 1.0)
    flag = np.where((mask == 0.0) | (np.arange(S) == 0), 0.0, 1.0).astype(f32)

    xaugs = []
    for c in range(NCORES):
        xc = x[c * BL:(c + 1) * BL].astype(f32)        # [BL, S, F]
        arr = np.empty((S, KX, BL), f32)
        arr[:, 0, :] = xc[:, :, 0].T
        arr[:, 1:F, :] = xc[:, :, 1:].transpose(1, 2, 0)
        arr[:, F, :] = flag[:, None]
        arr[:, F + 1, :] = 1.0
        a = arr.reshape(NBLK, U, KX, BL).transpose(0, 2, 1, 3)
        xaugs.append(np.ascontiguousarray(a.reshape(NBLK * KX, U * BL)))

    shared = {
        "whh0t": whh0t, "wih1t": wih1t, "whh1t": whh1t, "wih0t": wih0t,
        "bias1": np.ascontiguousarray(bias1), "woutt": woutt,
        "ones": np.ones((1, BL), f32), "zeros": np.zeros((128, 4 * BL), f32),
    }
    in_maps = [dict(shared, xaug=xaugs[c]) for c in range(NCORES)]
    return in_maps, float(np.asarray(bout).reshape(-1)[0])


def kernel(x, Wih0, Whh0, bih0, bhh0, Wih1, Whh1, bih1, bhh1, Wout, bout,
           _trace=False):
    global _LAST_RESULTS
    x = np.asarray(x)
    in_maps, bout_val = _prep_inputs(
        x, np.asarray(Wih0), np.asarray(Whh0), np.asarray(bih0),
        np.asarray(bhh0), np.asarray(Wih1), np.asarray(Whh1),
        np.asarray(bih1), np.asarray(bhh1), np.asarray(Wout),
        np.asarray(bout))
    nc = _build(bout_val, trace=_trace)
    res = run_bass_kernel_spmd(nc, in_maps, core_ids=list(range(NCORES)),
                               trace=_trace)
    _LAST_RESULTS = res
    out = np.empty((B, S, 1), np.float32)
    for c in range(NCORES):
        oc = res.results[c]["out"].reshape(S, BL)     # [S, BL]
        out[c * BL:(c + 1) * BL, :, 0] = oc.T
    return out


# revision 5
# speedup vs baseline: 1.0952x; 1.0952x over previous
"""Trainium2 Bass kernel for nn_AlternatingForecastModel.

2-layer LSTM (H=512) over S=2688 steps, B=512. Odd weeks feed the model's
previous prediction back as input feature 0. Data-parallel over batch:
8 cores x 64 rows, weights replicated, scan local per core.

Per core/step (bl=64): gates[bl, 2048] accumulate in PSUM via bf16 matmuls
with lhsT = transposed activations (curT [34,64] / hT chunks [128,64]) and
rhs = pre-transposed weights streamed at N=512. Gate rows are host-reordered
to [i, f, o, g] so one sigmoid covers cols 0:1536. Elementwise (fp32) on
ACT/DVE in the batch-partition domain; h_new transposed back via PE
transposes into reused PSUM banks, evacuated as bf16; pred = wout . h1 via
M=1 matmuls giving predT [1, 64]. Emission order software-pipelines steps:
gates1's h1-part fills the layer-0 elementwise gap, the next step's gates0
fills the layer-1 gap.
"""

import numpy as np
import ml_dtypes

import concourse.bacc as bacc
import concourse.mybir as mybir
import concourse.tile as tile
from concourse.bass import ds
from concourse.bass_utils import run_bass_kernel_spmd
from concourse.masks import make_identity

FP32 = mybir.dt.float32
BF16 = mybir.dt.bfloat16
AF = mybir.ActivationFunctionType

B, S, F = 512, 2688, 32
H = 512
G = 4 * H
WEEK = 672
NCORES = 8
BL = B // NCORES          # 64 batch rows per core
KX = F + 2                # 34: [feat0, x1..x31, flag, ones(bias)]
U = 8                     # steps per sub-block (one x DMA)
STEPS_PER_IT = 2 * U      # 16
IT_PER_WEEK = WEEK // STEPS_PER_IT  # 42
NBLK = S // U             # 336
NIT = S // STEPS_PER_IT   # 168

_LAST_RESULTS = None


def _build(bout_val: float, trace: bool = False):
    nc = bacc.Bacc("TRN2")

    xaug_d = nc.declare_dram_parameter("xaug", [NBLK * KX, U * BL], BF16, isOutput=False)
    whh0t_d = nc.declare_dram_parameter("whh0t", [128, 4 * G], BF16, isOutput=False)
    wih1t_d = nc.declare_dram_parameter("wih1t", [128, 4 * G], BF16, isOutput=False)
    whh1t_d = nc.declare_dram_parameter("whh1t", [128, 4 * G], BF16, isOutput=False)
    wih0t_d = nc.declare_dram_parameter("wih0t", [KX, G], BF16, isOutput=False)
    bias1_d = nc.declare_dram_parameter("bias1", [1, G], BF16, isOutput=False)
    woutt_d = nc.declare_dram_parameter("woutt", [128, 4], BF16, isOutput=False)
    ones_d = nc.declare_dram_parameter("ones", [1, BL], BF16, isOutput=False)
    zeros_d = nc.declare_dram_parameter("zeros", [128, 4 * BL], BF16, isOutput=False)
    out_d = nc.declare_dram_parameter("out", [NIT, STEPS_PER_IT * BL], FP32, isOutput=True)

    # SBUF
    whh0t = nc.alloc_sbuf_tensor("whh0t_s", [128, 4 * G], BF16)
    wih1t = nc.alloc_sbuf_tensor("wih1t_s", [128, 4 * G], BF16)
    whh1t = nc.alloc_sbuf_tensor("whh1t_s", [128, 4 * G], BF16)
    wih0t = nc.alloc_sbuf_tensor("wih0t_s", [KX, G], BF16)
    bias1 = nc.alloc_sbuf_tensor("bias1_s", [1, G], BF16)
    woutt = nc.alloc_sbuf_tensor("woutt_s", [128, 4], BF16)
    ones = nc.alloc_sbuf_tensor("ones_s", [1, BL], BF16)
    ident = nc.alloc_sbuf_tensor("ident", [BL, BL], FP32)

    xbuf = [nc.alloc_sbuf_tensor(f"xbuf{a}", [KX, U * BL], BF16) for a in (0, 1)]
    h0T = [nc.alloc_sbuf_tensor(f"h0T{p}", [128, 4 * BL], BF16) for p in (0, 1)]
    h1T = [nc.alloc_sbuf_tensor(f"h1T{p}", [128, 4 * BL], BF16) for p in (0, 1)]
    c0 = nc.alloc_sbuf_tensor("c0", [BL, H], FP32)
    c1 = nc.alloc_sbuf_tensor("c1", [BL, H], FP32)
    sig0 = [nc.alloc_sbuf_tensor(f"sig0{p}", [BL, 3 * H], FP32) for p in (0, 1)]
    sig1 = [nc.alloc_sbuf_tensor(f"sig1{p}", [BL, 3 * H], FP32) for p in (0, 1)]
    tg0 = [nc.alloc_sbuf_tensor(f"tg0{p}", [BL, H], FP32) for p in (0, 1)]
    tg1 = [nc.alloc_sbuf_tensor(f"tg1{p}", [BL, H], FP32) for p in (0, 1)]
    tc0 = [nc.alloc_sbuf_tensor(f"tc0{p}", [BL, H], FP32) for p in (0, 1)]
    tc1 = [nc.alloc_sbuf_tensor(f"tc1{p}", [BL, H], FP32) for p in (0, 1)]
    ta = [nc.alloc_sbuf_tensor(f"ta{p}", [BL, H], FP32) for p in (0, 1)]
    tb = [nc.alloc_sbuf_tensor(f"tb{p}", [BL, H], FP32) for p in (0, 1)]
    h0 = [nc.alloc_sbuf_tensor(f"h0{p}", [BL, H], FP32) for p in (0, 1)]
    h1 = [nc.alloc_sbuf_tensor(f"h1{p}", [BL, H], FP32) for p in (0, 1)]
    outst = nc.alloc_sbuf_tensor("outst", [1, STEPS_PER_IT * BL], FP32)
    predl = nc.alloc_sbuf_tensor("predl", [1, BL], BF16)

    g0 = nc.alloc_psum_tensor("g0", [128, G], FP32)
    g1 = nc.alloc_psum_tensor("g1", [128, G], FP32)

    NS = G // 512  # 4 N-slices per gate vector

    def emit_g0(u, xb):
        """gates0(t) = curT.T @ Wih0aug + h0_prev.T.T @ Whh0"""
        prev = (u + 1) % 2
        slot = (u % U) * BL
        xl = xb.ap()[0:KX, slot:slot + BL]
        for ns in range(NS):
            nc.tensor.matmul(g0.ap()[0:BL, ns * 512:(ns + 1) * 512],
                             xl, wih0t.ap()[:, ns * 512:(ns + 1) * 512],
                             start=True, stop=False)
        for k in range(4):
            lhs = h0T[prev].ap()[:, k * BL:(k + 1) * BL]
            for ns in range(NS):
                nc.tensor.matmul(
                    g0.ap()[0:BL, ns * 512:(ns + 1) * 512],
                    lhs, whh0t.ap()[:, k * G + ns * 512:k * G + (ns + 1) * 512],
                    start=False, stop=(k == 3))

    def emit_g1_part1(u):
        """bias1 + h1_prev-part of gates1(t)"""
        prev = (u + 1) % 2
        for ns in range(NS):
            nc.tensor.matmul(g1.ap()[0:BL, ns * 512:(ns + 1) * 512],
                             ones.ap(), bias1.ap()[:, ns * 512:(ns + 1) * 512],
                             start=True, stop=False)
        for k in range(4):
            lhs = h1T[prev].ap()[:, k * BL:(k + 1) * BL]
            for ns in range(NS):
                nc.tensor.matmul(
                    g1.ap()[0:BL, ns * 512:(ns + 1) * 512],
                    lhs, whh1t.ap()[:, k * G + ns * 512:k * G + (ns + 1) * 512],
                    start=False, stop=False)

    def emit_g1_part2(u):
        """h0_new-part of gates1(t)"""
        par = u % 2
        for k in range(4):
            lhs = h0T[par].ap()[:, k * BL:(k + 1) * BL]
            for ns in range(NS):
                nc.tensor.matmul(
                    g1.ap()[0:BL, ns * 512:(ns + 1) * 512],
                    lhs, wih1t.ap()[:, k * G + ns * 512:k * G + (ns + 1) * 512],
                    start=False, stop=(k == 3))

    def emit_ew(par, gps, sig, tg, tc, cc, hh):
        nc.scalar.activation(sig[par].ap(), gps.ap()[0:BL, 0:3 * H], AF.Sigmoid)
        nc.scalar.activation(tg[par].ap(), gps.ap()[0:BL, 3 * H:4 * H], AF.Tanh)
        nc.vector.tensor_mul(ta[par].ap(), sig[par].ap()[:, 0:H], tg[par].ap())
        nc.vector.tensor_mul(tb[par].ap(), sig[par].ap()[:, H:2 * H], cc.ap())
        nc.vector.tensor_add(cc.ap(), ta[par].ap(), tb[par].ap())
        nc.scalar.activation(tc[par].ap(), cc.ap(), AF.Tanh)
        nc.vector.tensor_mul(hh[par].ap(), sig[par].ap()[:, 2 * H:3 * H],
                             tc[par].ap())

    def emit_transpose(par, gps, hh, hT):
        for k in range(4):
            nc.tensor.transpose(gps.ap()[0:128, k * BL:(k + 1) * BL],
                                hh[par].ap()[0:BL, k * 128:(k + 1) * 128],
                                ident.ap())
        nc.vector.tensor_copy(hT[par].ap(), gps.ap()[0:128, 0:4 * BL])

    def emit_tail(u, pred_week, pred_dst, bout_val):
        """transpose h1, pred, output staging for step u."""
        par = u % 2
        emit_transpose(par, g1, h1, h1T)
        pps = g1.ap()[0:1, 512:512 + BL]
        for k in range(4):
            nc.tensor.matmul(pps, woutt.ap()[:, k:k + 1],
                             h1T[par].ap()[:, k * BL:(k + 1) * BL],
                             start=(k == 0), stop=(k == 3))
        nc.vector.tensor_scalar_add(outst.ap()[0:1, u * BL:(u + 1) * BL], pps,
                                    bout_val)
        if pred_week:
            dst_t, dst_col = pred_dst
            nc.vector.tensor_scalar_add(dst_t.ap()[0:1, dst_col:dst_col + BL],
                                        pps, bout_val)
        else:
            nc.vector.tensor_scalar_add(predl.ap(), pps, bout_val)

    with tile.TileContext(nc) as tc:
        # ---- preamble: weights, constants, state init ----
        make_identity(nc, ident.ap())
        nc.gpsimd.memset(c0.ap(), 0.0)
        nc.gpsimd.memset(c1.ap(), 0.0)
        nc.sync.dma_start(out=whh0t.ap(), in_=whh0t_d.ap())
        nc.sync.dma_start(out=wih1t.ap(), in_=wih1t_d.ap())
        nc.sync.dma_start(out=whh1t.ap(), in_=whh1t_d.ap())
        nc.sync.dma_start(out=wih0t.ap(), in_=wih0t_d.ap())
        nc.sync.dma_start(out=bias1.ap(), in_=bias1_d.ap())
        nc.sync.dma_start(out=woutt.ap(), in_=woutt_d.ap())
        nc.sync.dma_start(out=ones.ap(), in_=ones_d.ap())
        for p in (0, 1):
            nc.sync.dma_start(out=h0T[p].ap(), in_=zeros_d.ap())
            nc.sync.dma_start(out=h1T[p].ap(), in_=zeros_d.ap())

        def week_loop(week, pred_week):
            blk_base = week * WEEK // U
            it_base = week * WEEK // STEPS_PER_IT

            def body(i):
                for a in (0, 1):
                    if pred_week:
                        nc.sync.dma_start(
                            out=xbuf[a].ap()[1:KX, :],
                            in_=xaug_d.ap()[ds((blk_base + 2 * i + a) * KX + 1,
                                               KX - 1), :])
                    else:
                        nc.sync.dma_start(
                            out=xbuf[a].ap()[0:KX, :],
                            in_=xaug_d.ap()[ds((blk_base + 2 * i + a) * KX,
                                               KX), :])
                for u in range(STEPS_PER_IT):
                    par = u % 2
                    xb = xbuf[u // U]
                    emit_g0(u, xb)
                    if u > 0:
                        # tail of the previous step runs on PE while this
                        # step's layer-0 elementwise occupies ACT/DVE
                        up = u - 1
                        if pred_week:
                            if (up % U) < U - 1:
                                pdst = (xbuf[up // U], ((up % U) + 1) * BL)
                            else:
                                pdst = (xbuf[1 - up // U], 0)
                            emit_tail(up, True, pdst, bout_val)
                        else:
                            emit_tail(up, False, None, bout_val)
                    emit_g1_part1(u)
                    emit_ew(par, g0, sig0, tg0, tc0, c0, h0)
                    emit_transpose(par, g0, h0, h0T)
                    emit_g1_part2(u)
                    emit_ew(par, g1, sig1, tg1, tc1, c1, h1)
                u = STEPS_PER_IT - 1
                if pred_week:
                    emit_tail(u, True, (xbuf[0], 0), bout_val)
                else:
                    emit_tail(u, False, None, bout_val)
                nc.sync.dma_start(out=out_d.ap()[ds(it_base + i, 1), :],
                                  in_=outst.ap())

            with tc.For_i(0, IT_PER_WEEK, 1,
                          hint_engines=(mybir.EngineType.PE,
                                        mybir.EngineType.Activation,
                                        mybir.EngineType.DVE)) as i:
                body(i)

        week_loop(0, False)
        # pred(671) -> feat0 slot for t=672
        nc.vector.tensor_copy(xbuf[0].ap()[0:1, 0:BL], predl.ap())
        week_loop(1, True)
        week_loop(2, False)
        nc.vector.tensor_copy(xbuf[0].ap()[0:1, 0:BL], predl.ap())
        week_loop(3, True)

    nc.compile()
    return nc


def _prep_inputs(x, Wih0, Whh0, bih0, bhh0, Wih1, Whh1, bih1, bhh1, Wout, bout):
    """Host-side reshapes: gate reorder to [i,f,o,g], weight transposes,
    per-core xaug staging layout. All matmul operands cast to bf16."""
    f32 = np.float32
    bf16 = ml_dtypes.bfloat16
    perm = np.concatenate([np.arange(0, 512), np.arange(512, 1024),
                           np.arange(1536, 2048), np.arange(1024, 1536)])

    def wT(w):  # [G, 512] -> [128, 4*G] chunk-k at cols [G*k, G*k+G)
        t = np.ascontiguousarray(w[perm].T.astype(f32))          # [512, G]
        return np.ascontiguousarray(
            t.reshape(4, 128, G).transpose(1, 0, 2).reshape(128, 4 * G)
        ).astype(bf16)

    whh0t = wT(Whh0)
    wih1t = wT(Wih1)
    whh1t = wT(Whh1)
    bias0 = (bih0 + bhh0)[perm].astype(f32)
    bias1 = (bih1 + bhh1)[perm].astype(f32)[None, :].astype(bf16)
    wih0p = Wih0[perm].astype(f32)                               # [G, 33]
    wih0t = np.concatenate([wih0p.T, bias0[None, :]], axis=0)    # [34, G]
    wih0t = np.ascontiguousarray(wih0t).astype(bf16)
    woutt = np.ascontiguousarray(Wout.reshape(4, 128).T.astype(f32)).astype(bf16)

    tw = np.arange(S) // WEEK
    mask = np.where((tw % 2 == 0) & ((tw + 1) * WEEK <= S), 0.0, 1.0)
    flag = np.where((mask == 0.0) | (np.arange(S) == 0), 0.0, 1.0).astype(f32)

    xaugs = []
    for c in range(NCORES):
        xc = x[c * BL:(c + 1) * BL].astype(f32)        # [BL, S, F]
        arr = np.empty((S, KX, BL), f32)
        arr[:, 0, :] = xc[:, :, 0].T
        arr[:, 1:F, :] = xc[:, :, 1:].transpose(1, 2, 0)
        arr[:, F, :] = flag[:, None]
        arr[:, F + 1, :] = 1.0
        a = arr.reshape(NBLK, U, KX, BL).transpose(0, 2, 1, 3)
        xaugs.append(np.ascontiguousarray(
            a.reshape(NBLK * KX, U * BL)).astype(bf16))

    shared = {
        "whh0t": whh0t, "wih1t": wih1t, "whh1t": whh1t, "wih0t": wih0t,
        "bias1": np.ascontiguousarray(bias1), "woutt": woutt,
        "ones": np.ones((1, BL), bf16), "zeros": np.zeros((128, 4 * BL), bf16),
    }
    in_maps = [dict(shared, xaug=xaugs[c]) for c in range(NCORES)]
    return in_maps, float(np.asarray(bout).reshape(-1)[0])


def kernel(x, Wih0, Whh0, bih0, bhh0, Wih1, Whh1, bih1, bhh1, Wout, bout,
           _trace=False):
    global _LAST_RESULTS
    x = np.asarray(x)
    in_maps, bout_val = _prep_inputs(
        x, np.asarray(Wih0), np.asarray(Whh0), np.asarray(bih0),
        np.asarray(bhh0), np.asarray(Wih1), np.asarray(Whh1),
        np.asarray(bih1), np.asarray(bhh1), np.asarray(Wout),
        np.asarray(bout))
    nc = _build(bout_val, trace=_trace)
    res = run_bass_kernel_spmd(nc, in_maps, core_ids=list(range(NCORES)),
                               trace=_trace)
    _LAST_RESULTS = res
    out = np.empty((B, S, 1), np.float32)
    for c in range(NCORES):
        oc = res.results[c]["out"].reshape(S, BL)     # [S, BL]
        out[c * BL:(c + 1) * BL, :, 0] = oc.T
    return out


# revision 6
# speedup vs baseline: 1.1899x; 1.0865x over previous
"""Trainium2 Bass kernel for nn_AlternatingForecastModel.

2-layer LSTM (H=512) over S=2688 steps, B=512. Odd weeks feed the model's
previous prediction back as input feature 0. Data-parallel over batch:
8 cores x 64 rows, weights replicated, scan local per core.

Per core/step (bl=64): gates[bl, 2048] accumulate in PSUM via bf16 matmuls
with lhsT = transposed activations (curT [35,64] / hT chunks [128,64]) and
rhs = pre-transposed weights streamed at N=512. Layer-0 gates live in PSUM
partitions 0-63 (banks 0-3), layer-1 gates in partitions 64-127 (banks 4-7):
matmuls of the two layers target disjoint PE column groups and execute
concurrently (col tiling), with emission interleaved to pair them. Gate rows
are host-reordered to [i, f, o, g] so one sigmoid covers cols 0:1536.
Biases enter as hi+lo bf16 ones-rows (exact to ~2^-17). Elementwise (fp32)
on ACT/DVE; h_new transposed back via PE transposes into reused PSUM banks,
evacuated as bf16; pred = wout . h1 via M=1 matmuls giving predT [1, 64].
Emission software-pipelines: gates1's h1-part pairs with gates0, the next
step's Whh0-part pairs with gates1's h0-part.
"""

import numpy as np
import ml_dtypes

import concourse.bacc as bacc
import concourse.mybir as mybir
import concourse.tile as tile
from concourse.bass import ds
from concourse.bass_utils import run_bass_kernel_spmd

FP32 = mybir.dt.float32
BF16 = mybir.dt.bfloat16
AF = mybir.ActivationFunctionType

B, S, F = 512, 2688, 32
H = 512
G = 4 * H
WEEK = 672
NCORES = 8
BL = B // NCORES          # 64 batch rows per core
KX = F + 3                # 35: [feat0, x1..x31, flag, ones_hi, ones_lo]
U = 8                     # steps per sub-block (one x DMA)
STEPS_PER_IT = 2 * U      # 16
IT_PER_WEEK = WEEK // STEPS_PER_IT  # 42
NBLK = S // U             # 336
NIT = S // STEPS_PER_IT   # 168

_LAST_RESULTS = None


def _interleave(la, lb):
    """Alternate emission of two thunk lists (A/B PE col groups)."""
    n = max(len(la), len(lb))
    for i in range(n):
        if i < len(lb):
            lb[i]()
        if i < len(la):
            la[i]()


def _build(bout_val: float, trace: bool = False):
    nc = bacc.Bacc("TRN2")

    xaug_d = nc.declare_dram_parameter("xaug", [NBLK * KX, U * BL], BF16, isOutput=False)
    whh0t_d = nc.declare_dram_parameter("whh0t", [128, 4 * G], BF16, isOutput=False)
    wih1t_d = nc.declare_dram_parameter("wih1t", [128, 4 * G], BF16, isOutput=False)
    whh1t_d = nc.declare_dram_parameter("whh1t", [128, 4 * G], BF16, isOutput=False)
    wih0t_d = nc.declare_dram_parameter("wih0t", [KX, G], BF16, isOutput=False)
    bias1_d = nc.declare_dram_parameter("bias1", [2, G], BF16, isOutput=False)
    woutt_d = nc.declare_dram_parameter("woutt", [128, 4], BF16, isOutput=False)
    ones_d = nc.declare_dram_parameter("ones", [2, BL], BF16, isOutput=False)
    zeros_d = nc.declare_dram_parameter("zeros", [128, 4 * BL], BF16, isOutput=False)
    identc_d = nc.declare_dram_parameter("identc", [128, BL], FP32, isOutput=False)
    out_d = nc.declare_dram_parameter("out", [NIT, STEPS_PER_IT * BL], FP32, isOutput=True)

    # SBUF
    whh0t = nc.alloc_sbuf_tensor("whh0t_s", [128, 4 * G], BF16)
    wih1t = nc.alloc_sbuf_tensor("wih1t_s", [128, 4 * G], BF16)
    whh1t = nc.alloc_sbuf_tensor("whh1t_s", [128, 4 * G], BF16)
    wih0t = nc.alloc_sbuf_tensor("wih0t_s", [KX, G], BF16)
    bias1 = nc.alloc_sbuf_tensor("bias1_s", [2, G], BF16)
    woutt = nc.alloc_sbuf_tensor("woutt_s", [128, 4], BF16)
    ones = nc.alloc_sbuf_tensor("ones_s", [2, BL], BF16)
    identc = nc.alloc_sbuf_tensor("identc_s", [128, BL], FP32)

    xbuf = [nc.alloc_sbuf_tensor(f"xbuf{a}", [KX, U * BL], BF16) for a in (0, 1)]
    h0T = [nc.alloc_sbuf_tensor(f"h0T{p}", [128, 4 * BL], BF16) for p in (0, 1)]
    h1T = [nc.alloc_sbuf_tensor(f"h1T{p}", [128, 4 * BL], BF16) for p in (0, 1)]
    # layer-0 elementwise state at partitions 0-63, layer-1 at 64-127
    c0 = nc.alloc_sbuf_tensor("c0", [BL, H], FP32)
    c1f = nc.alloc_sbuf_tensor("c1f", [128, H], FP32)
    sig0 = [nc.alloc_sbuf_tensor(f"sig0{p}", [BL, 3 * H], FP32) for p in (0, 1)]
    sig1 = [nc.alloc_sbuf_tensor(f"sig1{p}", [128, 3 * H], FP32) for p in (0, 1)]
    tg0 = [nc.alloc_sbuf_tensor(f"tg0{p}", [BL, H], FP32) for p in (0, 1)]
    tg1 = [nc.alloc_sbuf_tensor(f"tg1{p}", [128, H], FP32) for p in (0, 1)]
    tc0 = [nc.alloc_sbuf_tensor(f"tc0{p}", [BL, H], FP32) for p in (0, 1)]
    tc1 = [nc.alloc_sbuf_tensor(f"tc1{p}", [128, H], FP32) for p in (0, 1)]
    ta0 = [nc.alloc_sbuf_tensor(f"ta0{p}", [BL, H], FP32) for p in (0, 1)]
    tb0 = [nc.alloc_sbuf_tensor(f"tb0{p}", [BL, H], FP32) for p in (0, 1)]
    ta1 = [nc.alloc_sbuf_tensor(f"ta1{p}", [128, H], FP32) for p in (0, 1)]
    tb1 = [nc.alloc_sbuf_tensor(f"tb1{p}", [128, H], FP32) for p in (0, 1)]
    h0 = [nc.alloc_sbuf_tensor(f"h0{p}", [BL, H], FP32) for p in (0, 1)]
    h1 = [nc.alloc_sbuf_tensor(f"h1{p}", [128, H], FP32) for p in (0, 1)]
    outst = nc.alloc_sbuf_tensor("outst", [1, STEPS_PER_IT * BL], FP32)
    predl = nc.alloc_sbuf_tensor("predl", [1, BL], BF16)

    g0 = nc.alloc_psum_tensor("g0", [128, G], FP32)
    g1 = nc.alloc_psum_tensor("g1", [128, G], FP32)

    NS = G // 512  # 4 N-slices per gate vector

    # ---- thunk builders for PE matmul groups (A = layer0/parts 0-63,
    #      B = layer1/parts 64-127) ----

    def mm_whh0(u, first):
        """Whh0-part of gates0(t): start group on k==0 when `first`."""
        prev = (u + 1) % 2
        out = []
        for k in range(4):
            lhs = h0T[prev].ap()[:, k * BL:(k + 1) * BL]
            for ns in range(NS):
                def f(k=k, ns=ns, lhs=lhs):
                    nc.tensor.matmul(
                        g0.ap()[0:BL, ns * 512:(ns + 1) * 512],
                        lhs,
                        whh0t.ap()[:, k * G + ns * 512:k * G + (ns + 1) * 512],
                        start=(first and k == 0), stop=False)
                out.append(f)
        return out

    def mm_xside(u, xb):
        slot = (u % U) * BL
        xl = xb.ap()[0:KX, slot:slot + BL]
        out = []
        for ns in range(NS):
            def f(ns=ns):
                nc.tensor.matmul(g0.ap()[0:BL, ns * 512:(ns + 1) * 512],
                                 xl, wih0t.ap()[:, ns * 512:(ns + 1) * 512],
                                 start=False, stop=(ns == NS - 1))
            out.append(f)
        return out

    def mm_bias1_whh1(u):
        prev = (u + 1) % 2
        out = []
        for ns in range(NS):
            def f(ns=ns):
                nc.tensor.matmul(g1.ap()[64:128, ns * 512:(ns + 1) * 512],
                                 ones.ap(), bias1.ap()[:, ns * 512:(ns + 1) * 512],
                                 start=True, stop=False)
            out.append(f)
        for k in range(4):
            lhs = h1T[prev].ap()[:, k * BL:(k + 1) * BL]
            for ns in range(NS):
                def f(k=k, ns=ns, lhs=lhs):
                    nc.tensor.matmul(
                        g1.ap()[64:128, ns * 512:(ns + 1) * 512],
                        lhs,
                        whh1t.ap()[:, k * G + ns * 512:k * G + (ns + 1) * 512],
                        start=False, stop=False)
                out.append(f)
        return out

    def mm_wih1(u):
        par = u % 2
        out = []
        for k in range(4):
            lhs = h0T[par].ap()[:, k * BL:(k + 1) * BL]
            for ns in range(NS):
                def f(k=k, ns=ns, lhs=lhs):
                    nc.tensor.matmul(
                        g1.ap()[64:128, ns * 512:(ns + 1) * 512],
                        lhs,
                        wih1t.ap()[:, k * G + ns * 512:k * G + (ns + 1) * 512],
                        start=False, stop=(k == 3))
                out.append(f)
        return out

    # ---- elementwise ----

    def emit_ew0(par):
        nc.scalar.activation(sig0[par].ap(), g0.ap()[0:BL, 0:3 * H], AF.Sigmoid)
        nc.scalar.activation(tg0[par].ap(), g0.ap()[0:BL, 3 * H:4 * H], AF.Tanh)
        nc.vector.tensor_mul(ta0[par].ap(), sig0[par].ap()[:, 0:H], tg0[par].ap())
        nc.vector.tensor_mul(tb0[par].ap(), sig0[par].ap()[:, H:2 * H], c0.ap())
        nc.vector.tensor_add(c0.ap(), ta0[par].ap(), tb0[par].ap())
        nc.scalar.activation(tc0[par].ap(), c0.ap(), AF.Tanh)
        nc.vector.tensor_mul(h0[par].ap(), sig0[par].ap()[:, 2 * H:3 * H],
                             tc0[par].ap())

    def emit_ew1(par):
        s = sig1[par].ap()[64:128, :]
        t = tg1[par].ap()[64:128, :]
        c = c1f.ap()[64:128, :]
        a = ta1[par].ap()[64:128, :]
        b = tb1[par].ap()[64:128, :]
        tc = tc1[par].ap()[64:128, :]
        hh = h1[par].ap()[64:128, :]
        nc.scalar.activation(s, g1.ap()[64:128, 0:3 * H], AF.Sigmoid)
        nc.scalar.activation(t, g1.ap()[64:128, 3 * H:4 * H], AF.Tanh)
        nc.vector.tensor_mul(a, s[:, 0:H], t)
        nc.vector.tensor_mul(b, s[:, H:2 * H], c)
        nc.vector.tensor_add(c, a, b)
        nc.scalar.activation(tc, c, AF.Tanh)
        nc.vector.tensor_mul(hh, s[:, 2 * H:3 * H], tc)

    def emit_transpose0(par):
        for k in range(4):
            nc.tensor.transpose(g0.ap()[0:128, k * BL:(k + 1) * BL],
                                h0[par].ap()[0:BL, k * 128:(k + 1) * 128],
                                identc.ap()[0:BL, :])
        nc.vector.tensor_copy(h0T[par].ap(), g0.ap()[0:128, 0:4 * BL])

    def emit_transpose1(par):
        for k in range(4):
            nc.tensor.transpose(g1.ap()[0:128, k * BL:(k + 1) * BL],
                                h1[par].ap()[64:128, k * 128:(k + 1) * 128],
                                identc.ap()[64:128, :])
        nc.vector.tensor_copy(h1T[par].ap(), g1.ap()[0:128, 0:4 * BL])

    def emit_pred(u, pred_week, pred_dst):
        par = u % 2
        pps = g1.ap()[0:1, 512:512 + BL]
        for k in range(4):
            nc.tensor.matmul(pps, woutt.ap()[:, k:k + 1],
                             h1T[par].ap()[:, k * BL:(k + 1) * BL],
                             start=(k == 0), stop=(k == 3))
        nc.vector.tensor_scalar_add(outst.ap()[0:1, u * BL:(u + 1) * BL], pps,
                                    bout_val)
        if pred_week:
            dst_t, dst_col = pred_dst
            nc.vector.tensor_scalar_add(dst_t.ap()[0:1, dst_col:dst_col + BL],
                                        pps, bout_val)
        else:
            nc.vector.tensor_scalar_add(predl.ap(), pps, bout_val)

    def pred_dst_for(u, pred_week):
        if not pred_week:
            return None
        if (u % U) < U - 1:
            return (xbuf[u // U], ((u % U) + 1) * BL)
        if u < STEPS_PER_IT - 1:
            return (xbuf[1 - u // U], 0)
        return (xbuf[0], 0)

    with tile.TileContext(nc) as tc:
        # ---- preamble: weights, constants, state init ----
        nc.gpsimd.memset(c0.ap(), 0.0)
        nc.gpsimd.memset(c1f.ap(), 0.0)
        nc.sync.dma_start(out=whh0t.ap(), in_=whh0t_d.ap())
        nc.sync.dma_start(out=wih1t.ap(), in_=wih1t_d.ap())
        nc.sync.dma_start(out=whh1t.ap(), in_=whh1t_d.ap())
        nc.sync.dma_start(out=wih0t.ap(), in_=wih0t_d.ap())
        nc.sync.dma_start(out=bias1.ap(), in_=bias1_d.ap())
        nc.sync.dma_start(out=woutt.ap(), in_=woutt_d.ap())
        nc.sync.dma_start(out=ones.ap(), in_=ones_d.ap())
        nc.sync.dma_start(out=identc.ap(), in_=identc_d.ap())
        for p in (0, 1):
            nc.sync.dma_start(out=h0T[p].ap(), in_=zeros_d.ap())
            nc.sync.dma_start(out=h1T[p].ap(), in_=zeros_d.ap())

        def week_loop(week, pred_week):
            blk_base = week * WEEK // U
            it_base = week * WEEK // STEPS_PER_IT

            def body(i):
                for a in (0, 1):
                    lo = 1 if pred_week else 0
                    nc.sync.dma_start(
                        out=xbuf[a].ap()[lo:KX, :],
                        in_=xaug_d.ap()[ds((blk_base + 2 * i + a) * KX + lo,
                                           KX - lo), :])
                # prologue for step 0: gates0(0) fully + gates1(0) part 1
                _interleave(mm_whh0(0, True) + mm_xside(0, xbuf[0]),
                            mm_bias1_whh1(0))
                for u in range(STEPS_PER_IT):
                    par = u % 2
                    emit_ew0(par)
                    emit_transpose0(par)
                    # gates1(t) h0-part pairs with next step's Whh0-part
                    if u + 1 < STEPS_PER_IT:
                        _interleave(mm_whh0(u + 1, True), mm_wih1(u))
                    else:
                        _interleave([], mm_wih1(u))
                    emit_ew1(par)
                    emit_transpose1(par)
                    emit_pred(u, pred_week, pred_dst_for(u, pred_week))
                    # next step: x-side (after pred write) + gates1 part 1
                    if u + 1 < STEPS_PER_IT:
                        _interleave(mm_xside(u + 1, xbuf[(u + 1) // U]),
                                    mm_bias1_whh1(u + 1))
                nc.sync.dma_start(out=out_d.ap()[ds(it_base + i, 1), :],
                                  in_=outst.ap())

            with tc.For_i(0, IT_PER_WEEK, 1,
                          hint_engines=(mybir.EngineType.PE,
                                        mybir.EngineType.Activation,
                                        mybir.EngineType.DVE)) as i:
                body(i)

        week_loop(0, False)
        # pred(671) -> feat0 slot for t=672
        nc.vector.tensor_copy(xbuf[0].ap()[0:1, 0:BL], predl.ap())
        week_loop(1, True)
        week_loop(2, False)
        nc.vector.tensor_copy(xbuf[0].ap()[0:1, 0:BL], predl.ap())
        week_loop(3, True)

    nc.compile()
    return nc


def _prep_inputs(x, Wih0, Whh0, bih0, bhh0, Wih1, Whh1, bih1, bhh1, Wout, bout):
    """Host-side reshapes: gate reorder to [i,f,o,g], weight transposes,
    hi/lo bias split, per-core xaug staging layout. bf16 matmul operands."""
    f32 = np.float32
    bf16 = ml_dtypes.bfloat16
    perm = np.concatenate([np.arange(0, 512), np.arange(512, 1024),
                           np.arange(1536, 2048), np.arange(1024, 1536)])

    def wT(w):  # [G, 512] -> [128, 4*G] chunk-k at cols [G*k, G*k+G)
        t = np.ascontiguousarray(w[perm].T.astype(f32))          # [512, G]
        return np.ascontiguousarray(
            t.reshape(4, 128, G).transpose(1, 0, 2).reshape(128, 4 * G)
        ).astype(bf16)

    def hilo(v):  # [G] fp32 -> [2, G] bf16 rows summing to ~v
        hi = v.astype(bf16).astype(f32)
        lo = (v - hi).astype(bf16)
        return np.stack([hi.astype(bf16), lo], axis=0)

    whh0t = wT(Whh0)
    wih1t = wT(Wih1)
    whh1t = wT(Whh1)
    bias0 = hilo((bih0 + bhh0)[perm].astype(f32))                # [2, G] bf16
    bias1 = hilo((bih1 + bhh1)[perm].astype(f32))
    wih0p = Wih0[perm].astype(f32)                               # [G, 33]
    wih0t = np.concatenate([wih0p.T.astype(bf16), bias0], axis=0)  # [35, G]
    wih0t = np.ascontiguousarray(wih0t)
    woutt = np.ascontiguousarray(Wout.reshape(4, 128).T.astype(f32)).astype(bf16)

    tw = np.arange(S) // WEEK
    mask = np.where((tw % 2 == 0) & ((tw + 1) * WEEK <= S), 0.0, 1.0)
    flag = np.where((mask == 0.0) | (np.arange(S) == 0), 0.0, 1.0).astype(f32)

    xaugs = []
    for c in range(NCORES):
        xc = x[c * BL:(c + 1) * BL].astype(f32)        # [BL, S, F]
        arr = np.empty((S, KX, BL), f32)
        arr[:, 0, :] = xc[:, :, 0].T
        arr[:, 1:F, :] = xc[:, :, 1:].transpose(1, 2, 0)
        arr[:, F, :] = flag[:, None]
        arr[:, F + 1, :] = 1.0
        arr[:, F + 2, :] = 1.0
        a = arr.reshape(NBLK, U, KX, BL).transpose(0, 2, 1, 3)
        xaugs.append(np.ascontiguousarray(
            a.reshape(NBLK * KX, U * BL)).astype(bf16))

    shared = {
        "whh0t": whh0t, "wih1t": wih1t, "whh1t": whh1t, "wih0t": wih0t,
        "bias1": np.ascontiguousarray(bias1), "woutt": woutt,
        "ones": np.ones((2, BL), bf16), "zeros": np.zeros((128, 4 * BL), bf16),
        "identc": np.ascontiguousarray(
            np.tile(np.eye(BL, dtype=f32), (2, 1))),
    }
    in_maps = [dict(shared, xaug=xaugs[c]) for c in range(NCORES)]
    return in_maps, float(np.asarray(bout).reshape(-1)[0])


def kernel(x, Wih0, Whh0, bih0, bhh0, Wih1, Whh1, bih1, bhh1, Wout, bout,
           _trace=False):
    global _LAST_RESULTS
    x = np.asarray(x)
    in_maps, bout_val = _prep_inputs(
        x, np.asarray(Wih0), np.asarray(Whh0), np.asarray(bih0),
        np.asarray(bhh0), np.asarray(Wih1), np.asarray(Whh1),
        np.asarray(bih1), np.asarray(bhh1), np.asarray(Wout),
        np.asarray(bout))
    nc = _build(bout_val, trace=_trace)
    res = run_bass_kernel_spmd(nc, in_maps, core_ids=list(range(NCORES)),
                               trace=_trace)
    _LAST_RESULTS = res
    out = np.empty((B, S, 1), np.float32)
    for c in range(NCORES):
        oc = res.results[c]["out"].reshape(S, BL)     # [S, BL]
        out[c * BL:(c + 1) * BL, :, 0] = oc.T
    return out
